# revision 4
# baseline (speedup 1.0000x reference)
"""Segment-max (GridPooling) kernel for 8 trn2 NeuronCores.

Strategy (bucketed equal-length layout, bf16):
  Host: group points by cell; split each occupied segment into chunks of
  at most CMAX points, pad each chunk with duplicate points to an even
  length (duplicates are max-neutral); bucket chunks by padded length.
  Build 16 streams (8 cores x 2 partition halves; the 64 features live
  on partitions) that all share ONE global tile schedule: every stream
  holds exactly m_c = ceil(n_c/16) chunks of each length c (short
  buckets padded with dummy chunks), packed into SBUF tiles of TF slots
  at identical offsets, DRAM packed compactly with per-tile transfer
  sizes.  Equal-length chunks are contiguous, so the device needs only
  grouped tensor_reduce max [128, n, c] -> [128, n] per run -- no
  masks, no scan, no shuffle.  Everything is bf16: rounding is monotone
  so max commutes with it and the result is exactly bf16(f32 max)
  (|rel err| <= 2^-9, far below the 2e-2 gate).
  Device (per core): double-buffered DMA tiles in, 1-3 grouped reduces
  per tile on the vector engine (bf16 hits the 4x DVE perf mode),
  stream dense per-chunk maxima out.  DMA is the bottleneck; the kernel
  runs at the HBM/DMA roofline for 2-byte traffic.
  Host: scatter chunk maxima back to segment ids (maximum over chunks
  of the same segment), upcast to f32.
"""
import sys

if "/opt/trn_rl_repo" not in sys.path:
    sys.path.insert(0, "/opt/trn_rl_repo")

import numpy as np
import ml_dtypes

BF16 = np.dtype(ml_dtypes.bfloat16)

TF = 16384     # SBUF slots per tile
CMAX = 2048    # max chunk length (segments longer are split)
NCORES = 8
NSTREAMS = 16

_nc_cache = {}


def _build_nc(L, fills, oc_total, oct_max, runs_per_tile, reps=1):
    """fills[d]: slots of tile d (DRAM is packed: tile d at offset sum(fills[:d]));
    runs_per_tile: list (len ntiles) of lists of (f0, oc0, n, c)."""
    import concourse.bass as bass
    from concourse import mybir

    bf16 = mybir.dt.bfloat16
    nc = bass.Bass()
    ntiles = len(fills)
    x_ext = nc.declare_dram_parameter("x", [128, L], bf16, isOutput=False)
    o_ext = nc.declare_dram_parameter("o", [128, oc_total], bf16, isOutput=True)

    NB = 3
    offs = [0]
    for f in fills:
        offs.append(offs[-1] + f)
    # per-tile output col spans (global)
    spans = []
    for runs in runs_per_tile:
        c0 = runs[0][1]
        c1 = runs[-1][1] + runs[-1][2]
        spans.append((c0, c1))

    import contextlib
    ctx = contextlib.ExitStack()
    with ctx:
        xt = [ctx.enter_context(nc.sbuf_tensor(f"xt{i}", [128, TF], bf16)) for i in range(NB)]
        ot = [ctx.enter_context(nc.sbuf_tensor(f"ot{i}", [128, oct_max], bf16)) for i in range(2)]
        in_sems = [ctx.enter_context(nc.semaphore(f"in_sem{i}")) for i in range(NB)]
        out_sems = [ctx.enter_context(nc.semaphore(f"out_sem{i}")) for i in range(2)]
        v_sem = ctx.enter_context(nc.semaphore("v_sem"))
        block = ctx.enter_context(nc.Block())

        total = ntiles * reps

        def x_dma(s, i):
            d = i % ntiles
            s.dma_start(xt[i % NB][:, 0:fills[d]],
                        x_ext[:, offs[d]:offs[d + 1]]).then_inc(in_sems[i % NB], 16)

        @block.sync
        def _(s):
            for i in range(min(NB, total)):
                x_dma(s, i)
            for i in range(total):
                d = i % ntiles
                c0, c1 = spans[d]
                s.wait_ge(v_sem, i + 1)
                s.dma_start(o_ext[:, c0:c1], ot[i % 2][:, 0:c1 - c0]).then_inc(out_sems[i % 2], 16)
                if i + NB < total:
                    x_dma(s, i + NB)

        @block.vector
        def _(v):
            for i in range(total):
                d = i % ntiles
                c0, _c1 = spans[d]
                v.wait_ge(in_sems[i % NB], 16 * (i // NB + 1))
                if i >= 2:
                    v.wait_ge(out_sems[i % 2], 16 * (i // 2))
                runs = runs_per_tile[d]
                for ri, (f0, oc0, n, c) in enumerate(runs):
                    instr = v.tensor_reduce(
                        ot[i % 2][:, oc0 - c0:oc0 - c0 + n],
                        xt[i % NB][:, f0:f0 + n * c].rearrange("p (n c) -> p n c", c=c),
                        axis=mybir.AxisListType.X, op=mybir.AluOpType.max)
                    if ri == len(runs) - 1:
                        instr.then_inc(v_sem, 1)

    return nc


def _preprocess(sig, idx, S):
    """Sort/bucket/pack on host.  Returns in_maps + assembly plan."""
    N, D = sig.shape
    assert D == 64, f"kernel assumes D=64, got {D}"
    counts = np.bincount(idx, minlength=S)
    order = np.argsort(idx, kind="stable")
    cstart = np.zeros(S + 1, np.int64)
    np.cumsum(counts, out=cstart[1:])

    occ = np.flatnonzero(counts > 0)                  # occupied segments
    clen = counts[occ]

    # split into chunks of <= CMAX points, each padded to an even length
    nch = -(-clen // CMAX)
    chunk_sid = np.repeat(occ, nch)                   # segment id per chunk
    e = np.cumsum(nch)
    chunk_no = np.arange(len(chunk_sid), dtype=np.int64) - np.repeat(e - nch, nch)
    base = chunk_no * CMAX                            # first point of chunk
    rem = counts[chunk_sid] - base                    # points in this chunk
    raw = np.minimum(rem, CMAX)
    plen = np.maximum(2, ((raw + 1) // 2) * 2)        # padded-even length

    # bucket chunks by plen, ascending segment id within bucket
    bucket_order = np.lexsort((chunk_sid, plen))
    sid_b = chunk_sid[bucket_order]
    base_b = base[bucket_order]
    plen_b = plen[bucket_order]

    # per bucket: pad chunk list to a multiple of NSTREAMS with dummies
    seq_sid_rows, seq_base_rows, len_seq_parts = [], [], []
    for c in np.unique(plen_b):
        m = plen_b == c
        ids, bas = sid_b[m], base_b[m]
        n_c = len(ids)
        m_c = -(-n_c // NSTREAMS)
        pad = NSTREAMS * m_c - n_c
        ids = np.concatenate([ids, np.full(pad, -1, np.int64)])
        bas = np.concatenate([bas, np.zeros(pad, np.int64)])
        seq_sid_rows.append(ids.reshape(NSTREAMS, m_c))
        seq_base_rows.append(bas.reshape(NSTREAMS, m_c))
        len_seq_parts.append(np.full(m_c, c, np.int64))
    seg_seq = np.concatenate(seq_sid_rows, axis=1)    # [16, Gtot] seg ids / -1
    base_seq = np.concatenate(seq_base_rows, axis=1)  # [16, Gtot] chunk base
    len_seq = np.concatenate(len_seq_parts)           # [Gtot]
    Gtot = len_seq.shape[0]

    # shared tile packing (greedy, whole chunks per tile; DRAM packed
    # compactly -- tile d occupies [offs[d], offs[d]+fills[d]) with no gaps)
    slot_off = np.empty(Gtot, np.int64)   # compact DRAM slot offset
    sbuf_off = np.empty(Gtot, np.int64)   # offset within the SBUF tile
    tile_id = np.empty(Gtot, np.int64)
    fills = []
    tile, fill, off = 0, 0, 0
    for j in range(Gtot):
        c = int(len_seq[j])
        if fill + c > TF:
            fills.append(fill)
            tile += 1
            fill = 0
        slot_off[j] = off
        sbuf_off[j] = fill
        tile_id[j] = tile
        fill += c
        off += c
    fills.append(fill)
    ntiles = tile + 1
    L = off

    # runs: consecutive j with same (tile, c)
    runs_per_tile = [[] for _ in range(ntiles)]
    j = 0
    while j < Gtot:
        t, c = int(tile_id[j]), int(len_seq[j])
        k = j
        while k < Gtot and tile_id[k] == t and len_seq[k] == c:
            k += 1
        runs_per_tile[t].append((int(sbuf_off[j]), j, k - j, c))
        j = k
    oct_max = max(r[-1][1] + r[-1][2] - r[0][1] for r in runs_per_tile)
    oc_total = Gtot

    # per-stream permutation -> X
    sig_t = sig.T.astype(BF16)                        # [64, N] bf16
    in_maps = []
    for core in range(NCORES):
        X = np.zeros((128, L), BF16)
        for h in range(2):
            k = 2 * core + h
            sj = seg_seq[k]
            valid = sj >= 0
            sid = sj[valid]
            lens = len_seq[valid]
            st = slot_off[valid]
            cb = base_seq[k][valid]
            e2 = np.cumsum(lens)
            pos = np.arange(int(e2[-1]), dtype=np.int64) - np.repeat(e2 - lens, lens)
            big_sid = np.repeat(sid, lens)
            src = cstart[big_sid] + np.minimum(np.repeat(cb, lens) + pos,
                                               counts[big_sid] - 1)
            dest = np.repeat(st, lens) + pos
            perm = np.zeros(L, np.int64)
            perm[dest] = order[src]
            X[64 * h:64 * (h + 1), :] = sig_t[:, perm]
        in_maps.append({"x": X})

    return in_maps, seg_seq, L, fills, oc_total, oct_max, runs_per_tile


def kernel(signal, cell_idx, num_segments):
    from concourse.bass_utils import run_bass_kernel_spmd

    sig = np.asarray(signal, dtype=np.float32)
    idx = np.asarray(cell_idx).astype(np.int64)
    S = int(num_segments)

    in_maps, seg_seq, L, fills, oc_total, oct_max, runs_per_tile = _preprocess(sig, idx, S)

    key = (L, tuple(fills), oc_total, oct_max,
           tuple(tuple(r) for t in runs_per_tile for r in t))
    if key not in _nc_cache:
        _nc_cache.clear()
        _nc_cache[key] = _build_nc(L, fills, oc_total, oct_max, runs_per_tile)
    nc = _nc_cache[key]

    res = run_bass_kernel_spmd(nc, in_maps, core_ids=list(range(NCORES)))

    multi_chunk = bool((np.bincount(seg_seq[seg_seq >= 0]) > 1).any())
    out = np.full((S, sig.shape[1]), -np.inf, np.float32)
    for core in range(NCORES):
        o = res.results[core]["o"]                    # [128, oc_total] bf16
        for h in range(2):
            k = 2 * core + h
            sj = seg_seq[k]
            valid = sj >= 0
            vals = o[64 * h:64 * (h + 1), valid].T.astype(np.float32)
            if multi_chunk:
                np.maximum.at(out, sj[valid], vals)
            else:
                out[sj[valid]] = vals
    return out
